# revision 20
# baseline (speedup 1.0000x reference)
"""GridMask kernel for Trainium2 (8 NeuronCores, batch-sharded SPMD).

out[n,c,s,h,w] = x[n,c,s,h,w] * mask[n,s,h,w], mask = row_hit OR col_hit
(per-(n,s) stripe predicates on h / w). Each core handles one batch element.

The f32 baseline streamed all 48MiB/core through SBUF (load+multiply+store),
saturating the 16 SDMA engines (~27 GB/s each, ~435 GB/s/core aggregate
shared by loads AND stores). Two observations cut SDMA engine-bytes ~2.6x:

  1. bf16: the harness gate is rel_err < 2e-2; casting x to bf16 on the host
     costs ~1.7e-3 relative error. All device traffic is bf16.
  2. Rows with row_hit=1 (~50%) have mask == 1 across the whole row: out
     row == x row. A direct HBM->HBM DMA moves those bytes through an SDMA
     engine ONCE instead of twice (load+store), never touching SBUF or the
     VectorEngine. The host permutes each (n,s) slab's rows so the first
     Rc rows are pure-copy rows (Rc = min over slabs of the copy-row count,
     rounded down to a multiple of 32 so tiles stay uniform; excess copy
     rows ride the mask path with flag=1, which is exact). The host
     un-permutes the output.

Mask path: the Rm=512-Rc masked rows of all 16 slices are packed flat per
channel (S*Rm rows, a multiple of 512) and processed as [128,4,512] tiles
(p-major, 4KB contiguous per-partition DMA runs). A flat row r belongs to
slice s = r // Rm -- static. Each tile's mask is built by the (idle)
TensorEngine into PSUM with ONE matmul per k-slot whose contraction stacks
the tile's <=3 constant-s segments:
    lhsT rows per segment i: [flag*g_i, g_i, -flag*g_i]   (g_i = partition
    indicator of segment i), rhs rows: [1, col_{s_i}, col_{s_i}]
so mask = flag OR col everywhere with a single base-0 full-width matmul.
Copy path: flat contiguous HBM->HBM chunks interleaved across the two HWDGE
rings (sync carries loads, scalar carries stores; sync gets more chunks
since loads finish earlier than stores).
"""

import math

import numpy as np

# problem shapes (hardcoded per harness contract)
N, C, S, H, W = 8, 3, 16, 512, 512
RATIO = 0.5
HH = math.ceil(math.sqrt(H * H + W * W))
OFF_H = (HH - H) // 2
OFF_W = (HH - W) // 2
P = 128
KK = 8  # rows per partition per tile (p-major); 8KB contiguous DMA runs
KH = KK // 2  # mask PSUM is built in two 4-bank halves
TR = P * KK  # 1024 rows per tile
NCORES = 8
NCHUNK = 8  # HBM->HBM copy chunks (issued after all tile work: tail filler)
SYNC_CHUNKS = 4  # chunks on the sync ring (rest on scalar)

_compiled = None
_compiled_rm = None


def _segments(t, Rm):
    """(lo, hi, s) row-offset segments of constant s inside tile t."""
    segs = []
    a, end = TR * t, TR * t + TR
    while a < end:
        s = a // Rm
        b = min((s + 1) * Rm, end)
        segs.append((a - TR * t, b - TR * t, s))
        a = b
    return segs


def _kdim(Rm):
    T = S * Rm // TR
    return 3 * max(len(_segments(t, Rm)) for t in range(T))


def _build(Rm):
    import concourse.bacc as bacc
    import concourse.mybir as mybir
    from concourse.mybir import AluOpType
    from concourse.tile import TileContext

    Rc = 512 - Rm
    T = S * Rm // TR  # [128,KK,W] tiles per channel
    KDIM = _kdim(Rm)
    copy_elems = C * S * Rc * W
    chunk = copy_elems // NCHUNK if copy_elems else 0

    nc = bacc.Bacc()
    xm = nc.dram_tensor("xm", [C, S * Rm, W], mybir.dt.bfloat16, kind="ExternalInput")
    lhsT = nc.dram_tensor("lhsT", [KDIM, T, KK, P], mybir.dt.bfloat16, kind="ExternalInput")
    rhs = nc.dram_tensor("rhs", [KDIM, T, W], mybir.dt.bfloat16, kind="ExternalInput")
    out_m = nc.dram_tensor("out_m", [C, S * Rm, W], mybir.dt.bfloat16, kind="ExternalOutput")
    if Rc:
        xc = nc.dram_tensor("xc", [NCHUNK, chunk], mybir.dt.bfloat16, kind="ExternalInput")
        out_c = nc.dram_tensor("out_c", [NCHUNK, chunk], mybir.dt.bfloat16, kind="ExternalOutput")

    with TileContext(nc) as tc:
        with (
            tc.tile_pool(name="params", bufs=1) as params,
            tc.tile_pool(name="xp", bufs=8) as xp,
            tc.tile_pool(name="mp", bufs=8, space="PSUM") as mp,
        ):
            lhsT_sb = params.tile([KDIM, T, KK, P], mybir.dt.bfloat16)
            rhs_sb = params.tile([KDIM, T, W], mybir.dt.bfloat16)
            # SWDGE ring: keeps the sync HWDGE FIFO free for the first loads
            nc.gpsimd.dma_start(out=lhsT_sb[:], in_=lhsT[:, :, :, :])
            nc.gpsimd.dma_start(out=rhs_sb[:], in_=rhs[:, :, :])
            for t in range(T):
                nseg = len(_segments(t, Rm))
                pmA = mp.tile([P, KH, W], mybir.dt.float32, bufs=1)
                pmB = mp.tile([P, KH, W], mybir.dt.float32, bufs=1)
                pms = [pmA, pmB]
                for j in range(KK):
                    nc.tensor.matmul(
                        pms[j // KH][:, j % KH, :],
                        lhsT_sb[: 3 * nseg, t, j, :],
                        rhs_sb[: 3 * nseg, t, :],
                        start=True,
                        stop=True,
                    )
                xt = xp.tile([P, C, KK, W], mybir.dt.bfloat16)
                for c in range(C):
                    nc.sync.dma_start(
                        out=xt[:, c],
                        in_=xm[c, TR * t : TR * (t + 1), :].rearrange(
                            "(p k) w -> p k w", p=P
                        ),
                    )
                for c in range(C):
                    for h in range(2):
                        nc.vector.tensor_tensor(
                            xt[:, c, h * KH : (h + 1) * KH, :],
                            xt[:, c, h * KH : (h + 1) * KH, :],
                            pms[h][:, :, :],
                            AluOpType.mult,
                        )
                    nc.scalar.dma_start(
                        out=out_m[c, TR * t : TR * (t + 1), :].rearrange(
                            "(p k) w -> p k w", p=P
                        ),
                        in_=xt[:, c],
                    )
            # dependency-free copy chunks queue behind all loads/stores: the
            # kernel tail becomes pure DMA drain with no engine idling
            if Rc:
                for i in range(NCHUNK):
                    eng = nc.sync if i < SYNC_CHUNKS else nc.scalar
                    eng.dma_start(out=out_c[i, :], in_=xc[i, :])
    nc.compile()
    return nc


def _hit_vectors(d, st_h, st_w):
    """row_hit [N,S,H] and col_hit [N,S,W] as bool."""
    d3 = d.astype(np.int64)[:, None, None]
    l3 = np.ceil(d.astype(np.float32) * RATIO).astype(np.int64)[:, None, None]
    sth = st_h.astype(np.int64) % d3[:, :, 0]
    stw = st_w.astype(np.int64) % d3[:, :, 0]
    rr = np.arange(H, dtype=np.int64)
    cc = np.arange(W, dtype=np.int64)
    row_hit = ((rr[None, None, :] + OFF_H - sth[:, :, None]) % d3) < l3
    col_hit = ((cc[None, None, :] + OFF_W - stw[:, :, None]) % d3) < l3
    return row_hit, col_hit


def _plan(d, st_h, st_w):
    """Row permutation + packed mask operands. Returns (Rm, perm, rowflag, colf)."""
    row_hit, col_hit = _hit_vectors(d, st_h, st_w)
    min_copy = int(row_hit.sum(axis=2).min())
    Rc = (min_copy // 64) * 64  # S*Rm must be a multiple of TR=1024
    Rm = 512 - Rc
    # stable sort: copy rows (row_hit True) first, preserving index order
    perm = np.argsort(~row_hit, axis=2, kind="stable").astype(np.int64)  # [N,S,H]
    flag = np.take_along_axis(row_hit, perm, axis=2)[:, :, Rc:]  # [N,S,Rm]
    return Rm, perm, flag.astype(np.float32), col_hit.astype(np.float32)


def _prep_in_maps(x, d, st_h, st_w):
    import ml_dtypes

    x = np.asarray(x, dtype=np.float32)
    d = np.asarray(d)
    st_h = np.asarray(st_h)
    st_w = np.asarray(st_w)
    Rm, perm, flag, colf = _plan(d, st_h, st_w)
    Rc = 512 - Rm
    T = S * Rm // TR
    KDIM = _kdim(Rm)

    xb = x.astype(ml_dtypes.bfloat16)  # [N,C,S,H,W]
    sidx = np.arange(S)[:, None]
    in_maps = []
    for n in range(N):
        g = xb[n][:, sidx, perm[n]]  # [C,S,512,W] rows permuted: copy-first
        f = flag[n].reshape(S * Rm)  # flat mask-path row flags
        # lhsT/rhs with K-stacked segments; flat row r = TR*t + KK*p + k
        fp = f.reshape(T, P, KK).transpose(0, 2, 1)  # [T,KK,P]
        lhsT = np.zeros((KDIM, T, KK, P), np.float32)
        rhs = np.zeros((KDIM, T, W), np.float32)
        for t in range(T):
            for i, (lo, hi, s) in enumerate(_segments(t, Rm)):
                gi = np.zeros(P, np.float32)
                gi[lo // KK : hi // KK] = 1.0
                lhsT[3 * i + 0, t] = fp[t] * gi
                lhsT[3 * i + 1, t] = gi
                lhsT[3 * i + 2, t] = -fp[t] * gi
                rhs[3 * i + 0, t] = 1.0
                rhs[3 * i + 1, t] = colf[n, s]
                rhs[3 * i + 2, t] = colf[n, s]
        m = {
            "xm": np.ascontiguousarray(g[:, :, Rc:]).reshape(C, S * Rm, W),
            "lhsT": lhsT.astype(ml_dtypes.bfloat16),
            "rhs": rhs.astype(ml_dtypes.bfloat16),
        }
        if Rc:
            m["xc"] = np.ascontiguousarray(g[:, :, :Rc]).reshape(NCHUNK, -1)
        in_maps.append(m)
    return in_maps


def kernel(x, d, st_h, st_w):
    from concourse.bass_utils import run_bass_kernel_spmd

    global _compiled, _compiled_rm
    x = np.asarray(x, dtype=np.float32)
    d = np.asarray(d)
    st_h = np.asarray(st_h)
    st_w = np.asarray(st_w)
    Rm, perm, _, _ = _plan(d, st_h, st_w)
    Rc = 512 - Rm
    if _compiled is None or _compiled_rm != Rm:
        _compiled = _build(Rm)
        _compiled_rm = Rm
    in_maps = _prep_in_maps(x, d, st_h, st_w)
    res = run_bass_kernel_spmd(_compiled, in_maps, core_ids=list(range(NCORES)))

    out = np.empty((N, C, S, H, W), dtype=np.float32)
    sidx = np.arange(S)[:, None]
    for n in range(N):
        r = res.results[n]
        permuted = np.empty((C, S, H, W), dtype=np.float32)
        if Rc:
            permuted[:, :, :Rc] = r["out_c"].reshape(C, S, Rc, W).astype(np.float32)
        permuted[:, :, Rc:] = r["out_m"].reshape(C, S, Rm, W).astype(np.float32)
        out[n][:, sidx, perm[n]] = permuted
    return out


# revision 21
# speedup vs baseline: 1.1862x; 1.1862x over previous
"""GridMask kernel for Trainium2 (8 NeuronCores, batch-sharded SPMD).

out[n,c,s,h,w] = x[n,c,s,h,w] * mask[n,s,h,w], mask = row_hit OR col_hit
(per-(n,s) stripe predicates on h / w). Each core handles one batch element.

The f32 baseline streamed all 48MiB/core through SBUF (load+multiply+store),
saturating the 16 SDMA engines (~27 GB/s each, ~435 GB/s/core aggregate
shared by loads AND stores). Two observations cut SDMA engine-bytes ~2.6x:

  1. bf16: the harness gate is rel_err < 2e-2; casting x to bf16 on the host
     costs ~1.7e-3 relative error. All device traffic is bf16.
  2. Rows with row_hit=1 (~50%) have mask == 1 across the whole row: out
     row == x row. A direct HBM->HBM DMA moves those bytes through an SDMA
     engine ONCE instead of twice (load+store), never touching SBUF or the
     VectorEngine. The host permutes each (n,s) slab's rows so the first
     Rc rows are pure-copy rows (Rc = min over slabs of the copy-row count,
     rounded down to a multiple of 32 so tiles stay uniform; excess copy
     rows ride the mask path with flag=1, which is exact). The host
     un-permutes the output.

Mask path: the Rm=512-Rc masked rows of all 16 slices are packed flat per
channel (S*Rm rows, a multiple of 512) and processed as [128,4,512] tiles
(p-major, 4KB contiguous per-partition DMA runs). A flat row r belongs to
slice s = r // Rm -- static. Each tile's mask is built by the (idle)
TensorEngine into PSUM with ONE matmul per k-slot whose contraction stacks
the tile's <=3 constant-s segments:
    lhsT rows per segment i: [flag*g_i, g_i, -flag*g_i]   (g_i = partition
    indicator of segment i), rhs rows: [1, col_{s_i}, col_{s_i}]
so mask = flag OR col everywhere with a single base-0 full-width matmul.
Copy path: flat contiguous HBM->HBM chunks interleaved across the two HWDGE
rings (sync carries loads, scalar carries stores; sync gets more chunks
since loads finish earlier than stores).
"""

import math

import numpy as np

# problem shapes (hardcoded per harness contract)
N, C, S, H, W = 8, 3, 16, 512, 512
RATIO = 0.5
HH = math.ceil(math.sqrt(H * H + W * W))
OFF_H = (HH - H) // 2
OFF_W = (HH - W) // 2
P = 128
KK = 8  # rows per partition per tile (p-major); 8KB contiguous DMA runs
KH = KK // 2  # mask PSUM is built in two 4-bank halves
TR = P * KK  # 1024 rows per tile
NCORES = 8
NCHUNK = 8  # HBM->HBM copy chunks (issued after all tile work: tail filler)
SYNC_CHUNKS = 4  # chunks on the sync ring (rest on scalar)

_compiled = None
_compiled_rm = None


def _segments(t, Rm):
    """(lo, hi, s) row-offset segments of constant s inside tile t."""
    segs = []
    a, end = TR * t, TR * t + TR
    while a < end:
        s = a // Rm
        b = min((s + 1) * Rm, end)
        segs.append((a - TR * t, b - TR * t, s))
        a = b
    return segs


def _kdim(Rm):
    T = S * Rm // TR
    return 3 * max(len(_segments(t, Rm)) for t in range(T))


def _build(Rm):
    import concourse.bacc as bacc
    import concourse.mybir as mybir
    from concourse.mybir import AluOpType
    from concourse.tile import TileContext

    Rc = 512 - Rm
    T = S * Rm // TR  # [128,KK,W] tiles per channel
    KDIM = _kdim(Rm)
    copy_elems = C * S * Rc * W
    chunk = copy_elems // NCHUNK if copy_elems else 0

    nc = bacc.Bacc()
    xm = nc.dram_tensor("xm", [C, S * Rm, W], mybir.dt.bfloat16, kind="ExternalInput")
    lhsT = nc.dram_tensor("lhsT", [KDIM, T, KK, P], mybir.dt.bfloat16, kind="ExternalInput")
    rhs = nc.dram_tensor("rhs", [KDIM, T, W], mybir.dt.bfloat16, kind="ExternalInput")
    out_m = nc.dram_tensor("out_m", [C, S * Rm, W], mybir.dt.bfloat16, kind="ExternalOutput")
    if Rc:
        xc = nc.dram_tensor("xc", [NCHUNK, chunk], mybir.dt.bfloat16, kind="ExternalInput")
        out_c = nc.dram_tensor("out_c", [NCHUNK, chunk], mybir.dt.bfloat16, kind="ExternalOutput")

    with TileContext(nc) as tc:
        with (
            tc.tile_pool(name="params", bufs=1) as params,
            tc.tile_pool(name="xp", bufs=8) as xp,
            tc.tile_pool(name="mp", bufs=8, space="PSUM") as mp,
        ):
            lhsT_sb = params.tile([KDIM, T, KK, P], mybir.dt.bfloat16)
            rhs_sb = params.tile([KDIM, T, W], mybir.dt.bfloat16)
            nc.sync.dma_start(out=lhsT_sb[:], in_=lhsT[:, :, :, :])
            nc.sync.dma_start(out=rhs_sb[:], in_=rhs[:, :, :])
            for t in range(T):
                nseg = len(_segments(t, Rm))
                pmA = mp.tile([P, KH, W], mybir.dt.float32, bufs=1)
                pmB = mp.tile([P, KH, W], mybir.dt.float32, bufs=1)
                pms = [pmA, pmB]
                for j in range(KK):
                    nc.tensor.matmul(
                        pms[j // KH][:, j % KH, :],
                        lhsT_sb[: 3 * nseg, t, j, :],
                        rhs_sb[: 3 * nseg, t, :],
                        start=True,
                        stop=True,
                    )
                xt = xp.tile([P, C, KK, W], mybir.dt.bfloat16)
                for c in range(C):
                    nc.sync.dma_start(
                        out=xt[:, c],
                        in_=xm[c, TR * t : TR * (t + 1), :].rearrange(
                            "(p k) w -> p k w", p=P
                        ),
                    )
                for c in range(C):
                    for h in range(2):
                        nc.vector.tensor_tensor(
                            xt[:, c, h * KH : (h + 1) * KH, :],
                            xt[:, c, h * KH : (h + 1) * KH, :],
                            pms[h][:, :, :],
                            AluOpType.mult,
                        )
                    nc.scalar.dma_start(
                        out=out_m[c, TR * t : TR * (t + 1), :].rearrange(
                            "(p k) w -> p k w", p=P
                        ),
                        in_=xt[:, c],
                    )
            # dependency-free copy chunks queue behind all loads/stores: the
            # kernel tail becomes pure DMA drain with no engine idling
            if Rc:
                for i in range(NCHUNK):
                    eng = nc.sync if i < SYNC_CHUNKS else nc.scalar
                    eng.dma_start(out=out_c[i, :], in_=xc[i, :])
    nc.compile()
    return nc


def _hit_vectors(d, st_h, st_w):
    """row_hit [N,S,H] and col_hit [N,S,W] as bool."""
    d3 = d.astype(np.int64)[:, None, None]
    l3 = np.ceil(d.astype(np.float32) * RATIO).astype(np.int64)[:, None, None]
    sth = st_h.astype(np.int64) % d3[:, :, 0]
    stw = st_w.astype(np.int64) % d3[:, :, 0]
    rr = np.arange(H, dtype=np.int64)
    cc = np.arange(W, dtype=np.int64)
    row_hit = ((rr[None, None, :] + OFF_H - sth[:, :, None]) % d3) < l3
    col_hit = ((cc[None, None, :] + OFF_W - stw[:, :, None]) % d3) < l3
    return row_hit, col_hit


def _plan(d, st_h, st_w):
    """Row permutation + packed mask operands. Returns (Rm, perm, rowflag, colf)."""
    row_hit, col_hit = _hit_vectors(d, st_h, st_w)
    min_copy = int(row_hit.sum(axis=2).min())
    Rc = (min_copy // 64) * 64  # S*Rm must be a multiple of TR=1024
    Rm = 512 - Rc
    # stable sort: copy rows (row_hit True) first, preserving index order
    perm = np.argsort(~row_hit, axis=2, kind="stable").astype(np.int64)  # [N,S,H]
    flag = np.take_along_axis(row_hit, perm, axis=2)[:, :, Rc:]  # [N,S,Rm]
    return Rm, perm, flag.astype(np.float32), col_hit.astype(np.float32)


def _prep_in_maps(x, d, st_h, st_w):
    import ml_dtypes

    x = np.asarray(x, dtype=np.float32)
    d = np.asarray(d)
    st_h = np.asarray(st_h)
    st_w = np.asarray(st_w)
    Rm, perm, flag, colf = _plan(d, st_h, st_w)
    Rc = 512 - Rm
    T = S * Rm // TR
    KDIM = _kdim(Rm)

    xb = x.astype(ml_dtypes.bfloat16)  # [N,C,S,H,W]
    sidx = np.arange(S)[:, None]
    in_maps = []
    for n in range(N):
        g = xb[n][:, sidx, perm[n]]  # [C,S,512,W] rows permuted: copy-first
        f = flag[n].reshape(S * Rm)  # flat mask-path row flags
        # lhsT/rhs with K-stacked segments; flat row r = TR*t + KK*p + k
        fp = f.reshape(T, P, KK).transpose(0, 2, 1)  # [T,KK,P]
        lhsT = np.zeros((KDIM, T, KK, P), np.float32)
        rhs = np.zeros((KDIM, T, W), np.float32)
        for t in range(T):
            for i, (lo, hi, s) in enumerate(_segments(t, Rm)):
                gi = np.zeros(P, np.float32)
                gi[lo // KK : hi // KK] = 1.0
                lhsT[3 * i + 0, t] = fp[t] * gi
                lhsT[3 * i + 1, t] = gi
                lhsT[3 * i + 2, t] = -fp[t] * gi
                rhs[3 * i + 0, t] = 1.0
                rhs[3 * i + 1, t] = colf[n, s]
                rhs[3 * i + 2, t] = colf[n, s]
        m = {
            "xm": np.ascontiguousarray(g[:, :, Rc:]).reshape(C, S * Rm, W),
            "lhsT": lhsT.astype(ml_dtypes.bfloat16),
            "rhs": rhs.astype(ml_dtypes.bfloat16),
        }
        if Rc:
            m["xc"] = np.ascontiguousarray(g[:, :, :Rc]).reshape(NCHUNK, -1)
        in_maps.append(m)
    return in_maps


def kernel(x, d, st_h, st_w):
    from concourse.bass_utils import run_bass_kernel_spmd

    global _compiled, _compiled_rm
    x = np.asarray(x, dtype=np.float32)
    d = np.asarray(d)
    st_h = np.asarray(st_h)
    st_w = np.asarray(st_w)
    Rm, perm, _, _ = _plan(d, st_h, st_w)
    Rc = 512 - Rm
    if _compiled is None or _compiled_rm != Rm:
        _compiled = _build(Rm)
        _compiled_rm = Rm
    in_maps = _prep_in_maps(x, d, st_h, st_w)
    res = run_bass_kernel_spmd(_compiled, in_maps, core_ids=list(range(NCORES)))

    out = np.empty((N, C, S, H, W), dtype=np.float32)
    sidx = np.arange(S)[:, None]
    for n in range(N):
        r = res.results[n]
        permuted = np.empty((C, S, H, W), dtype=np.float32)
        if Rc:
            permuted[:, :, :Rc] = r["out_c"].reshape(C, S, Rc, W).astype(np.float32)
        permuted[:, :, Rc:] = r["out_m"].reshape(C, S, Rm, W).astype(np.float32)
        out[n][:, sidx, perm[n]] = permuted
    return out
